# revision 48
# baseline (speedup 1.0000x reference)
"""BatchedGraphSAGEMean on 8 TRN2 NeuronCores.

Reference computation (per batch b of 8, N=2048 nodes, K=32 neighbors,
F_IN=256, F_OUT=256, CH=512):
    x_neib = mean_k x[idx[n,k]]                      [B,N,F]
    h = [x @ Wx^T + bx | x_neib @ Wn^T + bn]         [B,N,512]
    h = h / max(||h||_2(ch), 1e-12); h = relu(h)
    BatchNorm over (B,N) per channel (training stats, biased var, eps=1e-5)

Strategy (data-parallel over B, one batch per core):
  - The neighbor gather-mean is a matmul with a host-built adjacency count
    matrix AT[j, n] = #{k : idx[n,k] == j} (small ints, exact in fp8-e4m3).
    Using associativity:  h_nei = z^T-contract @ AT,  z = x @ (Wn^T/32),
    so the gather result needs no transpose.
  - Everything is computed channel-major (h^T [512, 2048]); the host
    transposes the output back. Channel-major makes the BatchNorm apply a
    single per-partition-scale/bias pass and BN stats come free via
    accum_out. The L2-norm reduce (over channels = partitions) goes
    through small ones-matmuls on the PE.
  - Single-pass bf16 matmuls and bf16 intermediates (the 2e-2 rel-err
    budget has ~5x headroom over the ~3e-3 this costs); BN sums stay f32.
  - Gather runs in two waves (blocks {0,1,2} then {3}) with each block's
    normalize/relu/BN-stat chain overlapped under the next blocks'
    matmuls, so only the last 512-node chain sits in the tail.
  - BN batch stats: per-core [128, 8] sums -> AllReduce over 8 cores.
    A tiny dummy AllReduce right after the loads absorbs the inter-core
    launch skew while the PE is still busy.
"""

import sys
import types

for _p in ("/opt/trn_rl_repo", "/root/.axon_site"):
    if _p not in sys.path:
        sys.path.append(_p)

import numpy as np
import ml_dtypes

import concourse.bass as bass
import concourse.bacc as bacc
import concourse.mybir as mybir
import concourse.tile as tile
from concourse.bass_utils import run_bass_kernel_spmd


def _install_ntff_hook_shim():
    """Make trace=True work under axon when antenv.axon_hooks is absent."""
    try:
        from antenv.axon_hooks import get_axon_ntff_profile_hook  # noqa: F401
        return
    except ImportError:
        pass
    try:
        import antenv
        from trn_agent_boot.trn_boot import _ntff_profile_via_ctypes
        hook = _ntff_profile_via_ctypes("/opt/axon/libaxon_pjrt.so")
        m = types.ModuleType("antenv.axon_hooks")
        m._hook = hook
        m.get_axon_ntff_profile_hook = lambda: m._hook
        m.set_axon_ntff_profile_hook = lambda h: setattr(m, "_hook", h)
        sys.modules["antenv.axon_hooks"] = m
        antenv.axon_hooks = m
    except Exception:
        pass


_install_ntff_hook_shim()

BF16 = mybir.dt.bfloat16
FP8 = mybir.dt.float8e4
F32 = mybir.dt.float32
A_DT = FP8            # adjacency counts <= 16 are exact in e4m3
AF = mybir.ActivationFunctionType
ALU = mybir.AluOpType

B, N, K, F, O = 8, 2048, 32, 256, 256
CH = 2 * O            # 512 channels
P = 128               # partitions
FC = F // P           # 2 f-chunks
OHALF = O // P        # 2 o-halves
NT = N // P           # 16 node tiles (z phase)
JC = N // P           # 16 source chunks (gather contraction)
JG = 8                # AT dma groups (2 jc each)
NB = 4                # node blocks
NBS = N // NB         # 512 nodes per block
NC_ = NBS // P        # 4 column chunks per block
PT = CH // P          # 4 channel partition-tiles
CORES = 8

# const-pack column layout
CPK_BIAS = 0
CPK_GAM = PT
CPK_BET = 2 * PT
CPK_EPS24 = 3 * PT          # 1e-24 column (norm guard)
CPK_EPSBN = 3 * PT + 1      # 1e-5 column (BN eps)
CPK_ONES = 3 * PT + 2
CPK_ID = 3 * PT + 2 + P
CPK_W = 3 * PT + 2 + 2 * P

_cache = {}


def build_program():
    nc = bacc.Bacc(None, target_bir_lowering=False)

    # ---- I/O (packed to minimize DMA trigger count) ----
    at_d = nc.declare_dram_parameter("AT", [N, N], A_DT, isOutput=False)
    xt_d = nc.declare_dram_parameter("xt", [F, N], BF16, isOutput=False)
    wpk_d = nc.declare_dram_parameter("wpk", [F, 2, O], BF16, isOutput=False)
    cpk_d = nc.declare_dram_parameter("cpk", [P, CPK_W], F32, isOutput=False)
    y_d = nc.declare_dram_parameter("y", [CH, N], BF16, isOutput=True)

    with tile.TileContext(nc) as tc:
        with (
            tc.tile_pool(name="big", bufs=1) as big,
            tc.tile_pool(name="consts", bufs=1) as consts,
            tc.tile_pool(name="htiles", bufs=16) as htiles,
            tc.tile_pool(name="work", bufs=8) as work,
            tc.tile_pool(name="sqp", bufs=16) as sqp,
            tc.tile_pool(name="rows", bufs=6) as rows,
            tc.tile_pool(name="yst", bufs=6) as yst,
            tc.tile_pool(name="smalls", bufs=1) as smalls,
            tc.tile_pool(name="ps", bufs=8, space="PSUM") as ps,
            tc.tile_pool(name="dram", bufs=4, space="DRAM") as dram,
        ):
            # ---- tiles ----
            atg = [big.tile([P, JC // JG, N], A_DT, name=f"atg{g}")
                   for g in range(JG)]
            xt = big.tile([P, FC, N], BF16)
            zh = big.tile([P, NT, O], BF16)
            g_sb = big.tile([P, PT, N], BF16)
            wpk = consts.tile([P, FC, 2, O], BF16)
            cpk = consts.tile([P, CPK_W], F32)
            cbf = consts.tile([P, 2 * P], BF16)   # bf16 ones | identity

            bias_c = cpk[:, CPK_BIAS:CPK_BIAS + PT]
            gam_c = cpk[:, CPK_GAM:CPK_GAM + PT]
            bet_c = cpk[:, CPK_BET:CPK_BET + PT]
            eps24_c = cpk[:, CPK_EPS24:CPK_EPS24 + 1]
            epsbn_c = cpk[:, CPK_EPSBN:CPK_EPSBN + 1]
            cones = cpk[:, CPK_ONES:CPK_ONES + P]
            cident = cpk[:, CPK_ID:CPK_ID + P]
            bones = cbf[:, 0:P]
            bident = cbf[:, P:2 * P]

            def wsl(fc, kind, oh=None):
                w = wpk[:, fc, kind, :]
                if oh is None:
                    return w
                return w[:, oh * P:(oh + 1) * P]

            # ---- loads: ONE queue, in consumption order, so the small
            # z/h_self inputs are not stuck in the rings behind 4MB of
            # adjacency; AT chunks then stream just-in-time for the jc loop
            nc.sync.dma_start(
                xt[:], xt_d[:].rearrange("(fc p) n -> p fc n", p=P))
            nc.sync.dma_start(
                wpk[:], wpk_d[:].rearrange("(fc p) a o -> p fc a o", p=P))
            nc.sync.dma_start(cpk[:], cpk_d[:])
            gsz = N // JG
            for g in range(JG):
                src = at_d[g * gsz:(g + 1) * gsz, :].rearrange(
                    "(a p) n -> p a n", p=P)
                nc.sync.dma_start(atg[g][:], src)

            # one-time touches: absorb the constant-DMA semaphores into the
            # DVE/ACT vector clocks so hot-loop instructions need at most one
            # wait (most instruction structs have a single wait slot).
            touch = smalls.tile([P, 2], F32)
            nc.vector.tensor_scalar(touch[:, 0:1], cpk[:, 0:1], 0.0, None,
                                    op0=ALU.add)
            nc.scalar.activation(touch[:, 1:2], cpk[:, 0:1], AF.Copy)
            # bf16 copies of the ones/identity consts (PE ldw operands)
            nc.scalar.activation(cbf[:, 0:P], cones[:], AF.Copy)
            nc.vector.tensor_copy(cbf[:, P:2 * P], cident[:])

            # early dummy AllReduce: pays the collective's fixed startup and
            # absorbs inter-core launch skew while the PE is still loading
            dum_in = dram.tile([P, 1], F32)
            dum_out = dram.tile([P, 1], F32)
            nc.scalar.dma_start(dum_in[:], cpk[:, 0:1])
            nc.gpsimd.collective_compute(
                "AllReduce", ALU.add,
                replica_groups=[list(range(CORES))],
                ins=[dum_in.opt()],
                outs=[dum_out.opt()],
            )



            # ---- phase 1: z = x @ (Wn^T/32), node-major bf16 ----
            for jt in range(NT):
                zp = ps.tile([P, O], F32, tag="pb", padded_shape=[P, NBS])
                njt = slice(jt * P, (jt + 1) * P)
                nc.tensor.matmul(zp[:], xt[:, 0, njt], wsl(0, 1),
                                 start=True, stop=False)
                nc.tensor.matmul(zp[:], xt[:, 1, njt], wsl(1, 1),
                                 start=False, stop=True)
                nc.vector.tensor_copy(zh[:, jt, :], zp[:])

            h_sb = {}
            sq = {}
            # NB+1 sum columns: block 3's chain runs as two half-chains
            sums_g = smalls.tile([P, PT, NB + 1], F32)
            sums_g2 = smalls.tile([P, PT, NB + 1], F32)

            # ---- phase 2: h_self^T = Wx @ x^T (ldw-amortized) ----
            hsp = {}
            for nb in range(NB):
                for oh in range(OHALF):
                    hsp[(nb, oh)] = ps.tile([P, NBS], F32, tag="pb",
                                            name=f"hsp{nb}_{oh}")
            for oh in range(OHALF):
                for fc in range(FC):
                    for nb in range(NB):
                        nsl = slice(nb * NBS, (nb + 1) * NBS)
                        nc.tensor.matmul(hsp[(nb, oh)][:], wsl(fc, 0, oh),
                                         xt[:, fc, nsl],
                                         start=(fc == 0), stop=(fc == FC - 1))
                for nb in range(NB):
                    pt = oh
                    ht = htiles.tile([P, NBS], BF16, tag="h",
                                     name=f"hts{nb}_{oh}")
                    h_sb[(pt, nb)] = ht
                    nc.scalar.activation(ht[:], hsp[(nb, oh)][:], AF.Identity,
                                         bias=bias_c[:, pt:pt + 1])

            # pre-squares for the h_self halves (DVE + GPSIMD, off the tail)
            for nb in range(NB):
                s0 = sqp.tile([P, NBS], BF16, tag="sq", name=f"sq{nb}_0")
                s1 = sqp.tile([P, NBS], BF16, tag="sq", name=f"sq{nb}_1")
                sq[(nb, 0)], sq[(nb, 1)] = s0, s1
                nc.vector.tensor_mul(s0[:], h_sb[(0, nb)][:], h_sb[(0, nb)][:])
                nc.gpsimd.tensor_mul(s1[:], h_sb[(1, nb)][:], h_sb[(1, nb)][:])

            # ---- gather waves:  h_nei^T = z^T-contract @ AT ----
            jpg = JC // JG
            def at_sl(jc, nsl):
                return atg[jc // jpg][:, jc % jpg, nsl]

            def gather_wave(nbs):
                """Both channel halves for node blocks `nbs`; ACT copies +
                DVE squares per half as they complete."""
                hnp = {(oh, nb): ps.tile([P, NBS], F32, tag="pb",
                                         name=f"hnp{oh}_{nb}")
                       for oh in range(OHALF) for nb in nbs}
                # jc-outer so each AT chunk is consumed once for BOTH
                # channel halves: the mm consumption rate then matches the
                # chunk arrival rate and the PE doesn't outrun the DMA
                for jc in range(JC):
                    for oh in range(OHALF):
                        osl = slice(oh * P, (oh + 1) * P)
                        for nb in nbs:
                            nsl = slice(nb * NBS, (nb + 1) * NBS)
                            nc.tensor.matmul(hnp[(oh, nb)][:],
                                             zh[:, jc, osl], at_sl(jc, nsl),
                                             start=(jc == 0),
                                             stop=(jc == JC - 1))
                for oh in range(OHALF):
                    for nb in nbs:
                        pt = 2 + oh
                        ht = htiles.tile([P, NBS], BF16, tag="h",
                                         name=f"htn{oh}_{nb}")
                        h_sb[(pt, nb)] = ht
                        nc.scalar.activation(ht[:], hnp[(oh, nb)][:],
                                             AF.Identity,
                                             bias=bias_c[:, pt:pt + 1])
                        sqt = sqp.tile([P, NBS], BF16, tag="sq",
                                       name=f"sq{nb}_{pt}")
                        sq[(nb, pt)] = sqt
                        nc.vector.tensor_mul(sqt[:], ht[:], ht[:])

            # chain(nb): per-node L2 norm -> rs -> g = relu(h*rs) with BN
            # sum/sumsq accumulation.  PE pieces (s2 ones-mms; the
            # column-spread / row-respread transposes) are emitted inside
            # the next wave's mm stream so the PE never idles long.
            chst = {}

            def chain_pe_s2(key, nb, c0, c1):
                w = c1 - c0
                s2row = ps.tile([P, NBS], F32, tag="pb", name=f"s2r{key}")
                for i in range(PT):
                    nc.tensor.matmul(s2row[0:1, 0:w], cbf[:, 0:1],
                                     sq[(nb, i)][:, c0:c1], start=(i == 0),
                                     stop=(i == PT - 1))
                s2sb = rows.tile([1, NBS], BF16, tag="srow",
                                 name=f"s2sb{key}")
                nc.scalar.activation(s2sb[:, 0:w], s2row[0:1, 0:w], AF.Copy)
                chst[key] = s2sb

            def chain_pe_pipe(key, w=NBS):
                s2sb = chst[key]
                ncw = w // P
                colsp = ps.tile([P, NBS], F32, tag="pb", name=f"colsp{key}")
                for c in range(ncw):
                    nc.tensor.matmul(colsp[:, c:c + 1],
                                     s2sb[0:1, c * P:(c + 1) * P],
                                     cbf[0:1, 0:1], start=True, stop=True)
                nrm = smalls.tile([P, ncw], F32, tag=f"nrm{key}")
                nc.scalar.activation(nrm[:], colsp[:, 0:ncw], AF.Sqrt,
                                     bias=eps24_c[:, :])
                rsc = smalls.tile([P, ncw], F32, tag=f"rsc{key}")
                nc.vector.reciprocal(rsc[:], nrm[:])
                rsT = ps.tile([P, NBS], F32, tag="pb", name=f"rsT{key}")
                for c in range(ncw):
                    nc.tensor.matmul(rsT[0:1, c * P:(c + 1) * P],
                                     rsc[:, c:c + 1], cident[:],
                                     start=True, stop=True)
                rrow = rows.tile([1, NBS], BF16, tag="rrow",
                                 name=f"rrow{key}")
                nc.scalar.activation(rrow[:, 0:w], rsT[0:1, 0:w], AF.Copy)
                rb = ps.tile([P, NBS], F32, tag="pb", name=f"rsb{key}")
                for c in range(ncw):
                    nc.tensor.matmul(rb[:, c * P:(c + 1) * P],
                                     bones[0:1, :],
                                     rrow[0:1, c * P:(c + 1) * P],
                                     start=True, stop=True)
                chst[key] = rb  # DVE reads the broadcast rs from PSUM

            def chain_vec(key, nb, c0, c1, scol):
                """g = relu(h)*rs (+BN sums) fused; g2 (+BN sumsq)."""
                rb = chst[key]
                w = c1 - c0
                for pt in range(PT):
                    gsl = g_sb[:, pt, nb * NBS + c0:nb * NBS + c1]
                    # relu commutes with the positive rs scale:
                    # g = max(h, 0) * rs, with the BN sum via accum.
                    # (scalar_tensor_tensor is DVE-only; Pool rejects it.)
                    nc.vector.scalar_tensor_tensor(
                        gsl, h_sb[(pt, nb)][:, c0:c1], 0.0, rb[:, 0:w],
                        op0=ALU.max, op1=ALU.mult,
                        accum_out=sums_g[:, pt, scol:scol + 1])
                    # g2 output is a throwaway (only accum_out matters)
                    g2t = work.tile([P, NBS], BF16, tag="wk",
                                    name=f"g2t{key}_{pt}")
                    if pt % 2 == 1:
                        nc.scalar.activation(
                            g2t[:, 0:w], gsl, AF.Square,
                            accum_out=sums_g2[:, pt, scol:scol + 1])
                    else:
                        nc.vector.scalar_tensor_tensor(
                            g2t[:, 0:w], gsl, 1.0, gsl, op0=ALU.mult,
                            op1=ALU.mult,
                            accum_out=sums_g2[:, pt, scol:scol + 1])

            # wave A: blocks 0..2 gather; chains emitted into wave B's stream
            gather_wave([0, 1, 2])

            # wave B: block 3, with wave-A chain PE ops interleaved so the
            # PE never stalls long on the vector chain
            hnpB = {oh: ps.tile([P, NBS], F32, tag="pb", name=f"hnpB{oh}")
                    for oh in range(OHALF)}
            nslB = slice(3 * NBS, 4 * NBS)
            for oh in range(OHALF):
                osl = slice(oh * P, (oh + 1) * P)
                for jc in range(JC):
                    nc.tensor.matmul(hnpB[oh][:], zh[:, jc, osl],
                                     at_sl(jc, nslB),
                                     start=(jc == 0), stop=(jc == JC - 1))
                    if oh == 0:
                        if jc == 4:
                            chain_pe_s2(0, 0, 0, NBS)
                        elif jc == 8:
                            chain_pe_s2(1, 1, 0, NBS)
                        elif jc == 10:
                            chain_pe_pipe(0)
                            chain_vec(0, 0, 0, NBS, 0)
                        elif jc == 13:
                            chain_pe_s2(2, 2, 0, NBS)
                        elif jc == 15:
                            chain_pe_pipe(1)
                            chain_vec(1, 1, 0, NBS, 1)
                    else:
                        if jc == 5:
                            chain_pe_pipe(2)
                            chain_vec(2, 2, 0, NBS, 2)
                # copies + squares for block 3's halves
                pt = 2 + oh
                ht = htiles.tile([P, NBS], BF16, tag="h", name=f"htn{oh}_3")
                h_sb[(pt, 3)] = ht
                nc.scalar.activation(ht[:], hnpB[oh][:], AF.Identity,
                                     bias=bias_c[:, pt:pt + 1])
                sqt = sqp.tile([P, NBS], BF16, tag="sq", name=f"sq3_{pt}")
                sq[(3, pt)] = sqt
                nc.vector.tensor_mul(sqt[:], ht[:], ht[:])

            # tail chain for block 3: two pipelined 256-col half-chains so
            # each serial stage is half-length
            H2 = NBS // 2
            chain_pe_s2("3a", 3, 0, H2)
            chain_pe_s2("3b", 3, H2, NBS)
            chain_pe_pipe("3a", H2)
            chain_vec("3a", 3, 0, H2, 3)
            chain_pe_pipe("3b", H2)
            chain_vec("3b", 3, H2, NBS, 4)

            # ---- phase 4: all-reduce BN stats ----
            stats_sb = smalls.tile([P, 2 * PT], F32)
            nc.vector.reduce_sum(stats_sb[:, 0:PT], sums_g[:],
                                 axis=mybir.AxisListType.X)
            nc.vector.reduce_sum(stats_sb[:, PT:2 * PT], sums_g2[:],
                                 axis=mybir.AxisListType.X)
            # AllGather + local tree-add: the gather is a single exchange
            # stage (vs the multi-stage mesh reduce) and the 3 adds are tiny
            stats_in = dram.tile([P, 2 * PT], F32)
            stats_ag = dram.tile([CORES * P, 2 * PT], F32)
            nc.sync.dma_start(stats_in[:], stats_sb[:])
            nc.gpsimd.collective_compute(
                "AllGather", ALU.bypass,
                replica_groups=[list(range(CORES))],
                ins=[stats_in.opt()],
                outs=[stats_ag.opt()],
            )
            allst = smalls.tile([P, CORES, 2 * PT], F32)
            nc.sync.dma_start(
                allst[:], stats_ag[:].rearrange("(c p) w -> p c w", p=P))
            r4 = smalls.tile([P, 4, 2 * PT], F32)
            nc.vector.tensor_tensor(r4[:], allst[:, 0:4, :],
                                    allst[:, 4:8, :], op=ALU.add)
            r2 = smalls.tile([P, 2, 2 * PT], F32)
            nc.vector.tensor_tensor(r2[:], r4[:, 0:2, :], r4[:, 2:4, :],
                                    op=ALU.add)
            allstats = smalls.tile([P, 2 * PT], F32)
            nc.vector.tensor_tensor(allstats[:], r2[:, 0, :], r2[:, 1, :],
                                    op=ALU.add)

            # mu/var -> per-channel scale/bias columns
            inv_n = 1.0 / (B * N)
            var_c = smalls.tile([P, PT], F32)
            s_c = smalls.tile([P, PT], F32)
            t_c = smalls.tile([P, PT], F32)
            mom_c = smalls.tile([P, 2 * PT], F32)
            nc.vector.tensor_scalar(mom_c[:], allstats[:], inv_n, None,
                                    op0=ALU.mult)
            mu_c = mom_c[:, 0:PT]
            ex2_c = mom_c[:, PT:2 * PT]
            nc.vector.tensor_mul(var_c[:], mu_c, mu_c)
            nc.vector.tensor_tensor(var_c[:], ex2_c, var_c[:],
                                    op=ALU.subtract)
            nc.scalar.activation(var_c[:], var_c[:], AF.Sqrt,
                                 bias=epsbn_c[:, :])
            nc.vector.reciprocal(var_c[:], var_c[:])
            nc.vector.tensor_mul(s_c[:], gam_c[:], var_c[:])
            nc.vector.tensor_mul(t_c[:], mu_c[:], s_c[:])
            nc.vector.tensor_tensor(t_c[:], bet_c[:], t_c[:], op=ALU.subtract)

            # ---- phase 5: y = g * s + t, store (wide tiles, 3 queues) ----
            N2 = N // 2
            dmaq = [nc.sync, nc.gpsimd, nc.scalar]
            k = 0
            for pt in range(PT):
                for half in range(2):
                    yt = yst.tile([P, N2], BF16, tag="y")
                    gsl = g_sb[:, pt, half * N2:(half + 1) * N2]
                    if k % 2 == 0:
                        nc.scalar.activation(
                            yt[:], gsl, AF.Identity, bias=t_c[:, pt:pt + 1],
                            scale=s_c[:, pt:pt + 1])
                    else:
                        nc.vector.tensor_scalar(
                            yt[:], gsl, s_c[:, pt:pt + 1], t_c[:, pt:pt + 1],
                            op0=ALU.mult, op1=ALU.add)
                    dmaq[k % 3].dma_start(
                        y_d[pt * P:(pt + 1) * P, half * N2:(half + 1) * N2],
                        yt[:])
                    k += 1

    nc.compile()
    return nc


def _prepare_inputs(x, idx_neib, Wx_w, Wx_b, Wn_w, Wn_b, gamma, beta):
    x = np.asarray(x, dtype=np.float32)
    idx = np.asarray(idx_neib, dtype=np.int64)
    Wx_w = np.asarray(Wx_w, dtype=np.float32)
    Wn_w = np.asarray(Wn_w, dtype=np.float32)
    Wx_b = np.asarray(Wx_b, dtype=np.float32)
    Wn_b = np.asarray(Wn_b, dtype=np.float32)
    gamma = np.asarray(gamma, dtype=np.float32)
    beta = np.asarray(beta, dtype=np.float32)

    # adjacency counts: AT[j, n] = #{k : idx[n, k] == j}
    at = np.zeros((N, N), dtype=np.float32)
    np.add.at(at, (idx.ravel(), np.repeat(np.arange(N), K)), 1.0)
    assert at.max() <= 16, "neighbor multiplicity too large for fp8 counts"
    at_q = at.astype(mybir.dt.np(A_DT))

    wx = np.ascontiguousarray(Wx_w.T).astype(ml_dtypes.bfloat16)
    wn = (np.ascontiguousarray(Wn_w.T) / np.float32(K)).astype(
        ml_dtypes.bfloat16)
    wpk = np.ascontiguousarray(np.stack([wx, wn], axis=1))

    cpk = np.zeros((P, CPK_W), dtype=np.float32)
    cpk[:, CPK_BIAS + 0] = Wx_b[0:P]
    cpk[:, CPK_BIAS + 1] = Wx_b[P:O]
    cpk[:, CPK_BIAS + 2] = Wn_b[0:P]
    cpk[:, CPK_BIAS + 3] = Wn_b[P:O]
    cpk[:, CPK_GAM:CPK_GAM + PT] = gamma.reshape(PT, P).T
    cpk[:, CPK_BET:CPK_BET + PT] = beta.reshape(PT, P).T
    cpk[:, CPK_EPS24] = 1e-24
    cpk[:, CPK_EPSBN] = 1e-5
    cpk[:, CPK_ONES:CPK_ONES + P] = 1.0
    cpk[:, CPK_ID:CPK_ID + P] = np.eye(P, dtype=np.float32)

    shared = dict(AT=at_q, wpk=wpk, cpk=cpk)
    in_maps = []
    for b in range(B):
        m = dict(shared)
        m["xt"] = np.ascontiguousarray(x[b].T).astype(ml_dtypes.bfloat16)
        in_maps.append(m)
    return in_maps


def kernel(x, idx_neib, Wx_w, Wx_b, Wn_w, Wn_b, gamma, beta, _trace=False,
           _trace_cores=None):
    if "nc" not in _cache:
        _cache["nc"] = build_program()
    nc = _cache["nc"]
    in_maps = _prepare_inputs(x, idx_neib, Wx_w, Wx_b, Wn_w, Wn_b, gamma, beta)
    res = run_bass_kernel_spmd(nc, in_maps, list(range(CORES)), trace=_trace,
                               trace_cores=_trace_cores)
    _cache["last_results"] = res
    y = np.stack([np.asarray(res.results[c]["y"]).astype(np.float32)
                  for c in range(CORES)])
    return np.ascontiguousarray(y.transpose(0, 2, 1))  # [B, N, CH]


# revision 53
# speedup vs baseline: 1.9438x; 1.9438x over previous
"""BatchedGraphSAGEMean on 8 TRN2 NeuronCores.

Reference computation (per batch b of 8, N=2048 nodes, K=32 neighbors,
F_IN=256, F_OUT=256, CH=512):
    x_neib = mean_k x[idx[n,k]]                      [B,N,F]
    h = [x @ Wx^T + bx | x_neib @ Wn^T + bn]         [B,N,512]
    h = h / max(||h||_2(ch), 1e-12); h = relu(h)
    BatchNorm over (B,N) per channel (training stats, biased var, eps=1e-5)

Strategy (data-parallel over B, one batch per core):
  - The neighbor gather-mean is a matmul with a host-built adjacency count
    matrix AT[j, n] = #{k : idx[n,k] == j} (small ints, exact in fp8-e4m3).
    Using associativity:  h_nei = z^T-contract @ AT,  z = x @ (Wn^T/32),
    so the gather result needs no transpose.
  - Everything is computed channel-major (h^T [512, 2048]); the host
    transposes the output back. Channel-major makes the BatchNorm apply a
    single per-partition-scale/bias pass and BN stats come free via
    accum_out. The L2-norm reduce (over channels = partitions) goes
    through small ones-matmuls on the PE.
  - Single-pass bf16 matmuls and bf16 intermediates (the 2e-2 rel-err
    budget has ~5x headroom over the ~3e-3 this costs); BN sums stay f32.
  - Gather runs in two waves (blocks {0,1,2} then {3}) with each block's
    normalize/relu/BN-stat chain overlapped under the next blocks'
    matmuls, so only the last 512-node chain sits in the tail.
  - BN batch stats: per-core [128, 8] sums -> AllReduce over 8 cores.
    A tiny dummy AllReduce right after the loads absorbs the inter-core
    launch skew while the PE is still busy.
"""

import sys
import types

for _p in ("/opt/trn_rl_repo", "/root/.axon_site"):
    if _p not in sys.path:
        sys.path.append(_p)

import numpy as np
import ml_dtypes

import concourse.bass as bass
import concourse.bacc as bacc
import concourse.mybir as mybir
import concourse.tile as tile
from concourse.bass_utils import run_bass_kernel_spmd


def _install_ntff_hook_shim():
    """Make trace=True work under axon when antenv.axon_hooks is absent."""
    try:
        from antenv.axon_hooks import get_axon_ntff_profile_hook  # noqa: F401
        return
    except ImportError:
        pass
    try:
        import antenv
        from trn_agent_boot.trn_boot import _ntff_profile_via_ctypes
        hook = _ntff_profile_via_ctypes("/opt/axon/libaxon_pjrt.so")
        m = types.ModuleType("antenv.axon_hooks")
        m._hook = hook
        m.get_axon_ntff_profile_hook = lambda: m._hook
        m.set_axon_ntff_profile_hook = lambda h: setattr(m, "_hook", h)
        sys.modules["antenv.axon_hooks"] = m
        antenv.axon_hooks = m
    except Exception:
        pass


_install_ntff_hook_shim()

BF16 = mybir.dt.bfloat16
FP8 = mybir.dt.float8e4
F32 = mybir.dt.float32
A_DT = FP8            # adjacency counts <= 16 are exact in e4m3
AF = mybir.ActivationFunctionType
ALU = mybir.AluOpType

B, N, K, F, O = 8, 2048, 32, 256, 256
CH = 2 * O            # 512 channels
P = 128               # partitions
FC = F // P           # 2 f-chunks
OHALF = O // P        # 2 o-halves
NT = N // P           # 16 node tiles (z phase)
JC = N // P           # 16 source chunks (gather contraction)
JG = 8                # AT dma groups (2 jc each)
NB = 4                # node blocks
NBS = N // NB         # 512 nodes per block
NC_ = NBS // P        # 4 column chunks per block
PT = CH // P          # 4 channel partition-tiles
CORES = 8

# const-pack column layout
CPK_BIAS = 0
CPK_GAM = PT
CPK_BET = 2 * PT
CPK_EPS24 = 3 * PT          # 1e-24 column (norm guard)
CPK_EPSBN = 3 * PT + 1      # 1e-5 column (BN eps)
CPK_ONES = 3 * PT + 2
CPK_ID = 3 * PT + 2 + P
CPK_W = 3 * PT + 2 + 2 * P

_cache = {}


def build_program():
    nc = bacc.Bacc(None, target_bir_lowering=False)

    # ---- I/O (packed to minimize DMA trigger count) ----
    at_d = nc.declare_dram_parameter("AT", [N, N], A_DT, isOutput=False)
    xt_d = nc.declare_dram_parameter("xt", [F, N], BF16, isOutput=False)
    wpk_d = nc.declare_dram_parameter("wpk", [F, 2, O], BF16, isOutput=False)
    cpk_d = nc.declare_dram_parameter("cpk", [P, CPK_W], F32, isOutput=False)
    y_d = nc.declare_dram_parameter("y", [CH, N], BF16, isOutput=True)

    with tile.TileContext(nc) as tc:
        with (
            tc.tile_pool(name="big", bufs=1) as big,
            tc.tile_pool(name="consts", bufs=1) as consts,
            tc.tile_pool(name="htiles", bufs=16) as htiles,
            tc.tile_pool(name="work", bufs=8) as work,
            tc.tile_pool(name="sqp", bufs=16) as sqp,
            tc.tile_pool(name="rows", bufs=6) as rows,
            tc.tile_pool(name="yst", bufs=6) as yst,
            tc.tile_pool(name="smalls", bufs=1) as smalls,
            tc.tile_pool(name="ps", bufs=8, space="PSUM") as ps,
            tc.tile_pool(name="dram", bufs=4, space="DRAM") as dram,
        ):
            # ---- tiles ----
            atg = [big.tile([P, JC // JG, N], A_DT, name=f"atg{g}")
                   for g in range(JG)]
            xt = big.tile([P, FC, N], BF16)
            zh = big.tile([P, NT, O], BF16)
            g_sb = big.tile([P, PT, N], BF16)
            wpk = consts.tile([P, FC, 2, O], BF16)
            cpk = consts.tile([P, CPK_W], F32)
            cbf = consts.tile([P, 2 * P], BF16)   # bf16 ones | identity

            bias_c = cpk[:, CPK_BIAS:CPK_BIAS + PT]
            gam_c = cpk[:, CPK_GAM:CPK_GAM + PT]
            bet_c = cpk[:, CPK_BET:CPK_BET + PT]
            eps24_c = cpk[:, CPK_EPS24:CPK_EPS24 + 1]
            epsbn_c = cpk[:, CPK_EPSBN:CPK_EPSBN + 1]
            cones = cpk[:, CPK_ONES:CPK_ONES + P]
            cident = cpk[:, CPK_ID:CPK_ID + P]
            bones = cbf[:, 0:P]
            bident = cbf[:, P:2 * P]

            def wsl(fc, kind, oh=None):
                w = wpk[:, fc, kind, :]
                if oh is None:
                    return w
                return w[:, oh * P:(oh + 1) * P]

            # ---- loads: ONE queue, in consumption order, so the small
            # z/h_self inputs are not stuck in the rings behind 4MB of
            # adjacency; AT chunks then stream just-in-time for the jc loop
            nc.sync.dma_start(
                xt[:], xt_d[:].rearrange("(fc p) n -> p fc n", p=P))
            nc.sync.dma_start(
                wpk[:], wpk_d[:].rearrange("(fc p) a o -> p fc a o", p=P))
            nc.sync.dma_start(cpk[:], cpk_d[:])
            gsz = N // JG
            for g in range(JG):
                src = at_d[g * gsz:(g + 1) * gsz, :].rearrange(
                    "(a p) n -> p a n", p=P)
                nc.sync.dma_start(atg[g][:], src)

            # one-time touches: absorb the constant-DMA semaphores into the
            # DVE/ACT vector clocks so hot-loop instructions need at most one
            # wait (most instruction structs have a single wait slot).
            touch = smalls.tile([P, 2], F32)
            nc.vector.tensor_scalar(touch[:, 0:1], cpk[:, 0:1], 0.0, None,
                                    op0=ALU.add)
            nc.scalar.activation(touch[:, 1:2], cpk[:, 0:1], AF.Copy)
            # bf16 copies of the ones/identity consts (PE ldw operands)
            nc.scalar.activation(cbf[:, 0:P], cones[:], AF.Copy)
            nc.vector.tensor_copy(cbf[:, P:2 * P], cident[:])

            # early dummy AllReduce: pays the collective's fixed startup and
            # absorbs inter-core launch skew while the PE is still loading
            dum_in = dram.tile([P, 1], F32)
            dum_out = dram.tile([P, 1], F32)
            nc.scalar.dma_start(dum_in[:], cpk[:, 0:1])
            nc.gpsimd.collective_compute(
                "AllReduce", ALU.add,
                replica_groups=[list(range(CORES))],
                ins=[dum_in.opt()],
                outs=[dum_out.opt()],
            )



            # ---- phase 1: z = x @ (Wn^T/32), node-major bf16 ----
            for jt in range(NT):
                zp = ps.tile([P, O], F32, tag="pb", padded_shape=[P, NBS])
                njt = slice(jt * P, (jt + 1) * P)
                nc.tensor.matmul(zp[:], xt[:, 0, njt], wsl(0, 1),
                                 start=True, stop=False)
                nc.tensor.matmul(zp[:], xt[:, 1, njt], wsl(1, 1),
                                 start=False, stop=True)
                nc.vector.tensor_copy(zh[:, jt, :], zp[:])

            h_sb = {}
            sq = {}
            # NB+1 sum columns: block 3's chain runs as two half-chains
            sums_g = smalls.tile([P, PT, NB + 1], F32)
            sums_g2 = smalls.tile([P, PT, NB + 1], F32)

            # ---- phase 2: h_self^T = Wx @ x^T (ldw-amortized) ----
            hsp = {}
            for nb in range(NB):
                for oh in range(OHALF):
                    hsp[(nb, oh)] = ps.tile([P, NBS], F32, tag="pb",
                                            name=f"hsp{nb}_{oh}")
            for oh in range(OHALF):
                for fc in range(FC):
                    for nb in range(NB):
                        nsl = slice(nb * NBS, (nb + 1) * NBS)
                        nc.tensor.matmul(hsp[(nb, oh)][:], wsl(fc, 0, oh),
                                         xt[:, fc, nsl],
                                         start=(fc == 0), stop=(fc == FC - 1))
                for nb in range(NB):
                    pt = oh
                    ht = htiles.tile([P, NBS], BF16, tag="h",
                                     name=f"hts{nb}_{oh}")
                    h_sb[(pt, nb)] = ht
                    nc.scalar.activation(ht[:], hsp[(nb, oh)][:], AF.Identity,
                                         bias=bias_c[:, pt:pt + 1])

            # pre-squares for the h_self halves (DVE + GPSIMD, off the tail)
            for nb in range(NB):
                s0 = sqp.tile([P, NBS], BF16, tag="sq", name=f"sq{nb}_0")
                s1 = sqp.tile([P, NBS], BF16, tag="sq", name=f"sq{nb}_1")
                sq[(nb, 0)], sq[(nb, 1)] = s0, s1
                nc.vector.tensor_mul(s0[:], h_sb[(0, nb)][:], h_sb[(0, nb)][:])
                nc.gpsimd.tensor_mul(s1[:], h_sb[(1, nb)][:], h_sb[(1, nb)][:])
                # pre-add the pair so the s2 ones-mm reads one tile
                nc.vector.tensor_add(s0[:], s0[:], s1[:])

            # ---- gather waves:  h_nei^T = z^T-contract @ AT ----
            jpg = JC // JG
            def at_sl(jc, nsl):
                return atg[jc // jpg][:, jc % jpg, nsl]

            def gather_wave(nbs):
                """Both channel halves for node blocks `nbs`; ACT copies +
                DVE squares per half as they complete."""
                hnp = {(oh, nb): ps.tile([P, NBS], F32, tag="pb",
                                         name=f"hnp{oh}_{nb}")
                       for oh in range(OHALF) for nb in nbs}
                # jc-outer so each AT chunk is consumed once for BOTH
                # channel halves: the mm consumption rate then matches the
                # chunk arrival rate and the PE doesn't outrun the DMA
                for jc in range(JC):
                    for oh in range(OHALF):
                        osl = slice(oh * P, (oh + 1) * P)
                        for nb in nbs:
                            nsl = slice(nb * NBS, (nb + 1) * NBS)
                            nc.tensor.matmul(hnp[(oh, nb)][:],
                                             zh[:, jc, osl], at_sl(jc, nsl),
                                             start=(jc == 0),
                                             stop=(jc == JC - 1))
                for oh in range(OHALF):
                    for nb in nbs:
                        pt = 2 + oh
                        ht = htiles.tile([P, NBS], BF16, tag="h",
                                         name=f"htn{oh}_{nb}")
                        h_sb[(pt, nb)] = ht
                        nc.scalar.activation(ht[:], hnp[(oh, nb)][:],
                                             AF.Identity,
                                             bias=bias_c[:, pt:pt + 1])
                        sqt = sqp.tile([P, NBS], BF16, tag="sq",
                                       name=f"sq{nb}_{pt}")
                        sq[(nb, pt)] = sqt
                        nc.vector.tensor_mul(sqt[:], ht[:], ht[:])
                        if pt == 3:
                            nc.vector.tensor_add(sq[(nb, 2)][:],
                                                 sq[(nb, 2)][:], sqt[:])

            # chain(nb): per-node L2 norm -> rs -> g = relu(h*rs) with BN
            # sum/sumsq accumulation.  PE pieces (s2 ones-mms; the
            # column-spread / row-respread transposes) are emitted inside
            # the next wave's mm stream so the PE never idles long.
            chst = {}

            def chain_pe_s2(key, nb, c0, c1):
                w = c1 - c0
                s2row = ps.tile([P, NBS], F32, tag="pb", name=f"s2r{key}")
                for i in range(2):
                    nc.tensor.matmul(s2row[0:1, 0:w], cbf[:, 0:1],
                                     sq[(nb, 2 * i)][:, c0:c1],
                                     start=(i == 0), stop=(i == 1))
                s2sb = rows.tile([1, NBS], BF16, tag="srow",
                                 name=f"s2sb{key}")
                nc.scalar.activation(s2sb[:, 0:w], s2row[0:1, 0:w], AF.Copy)
                chst[key] = s2sb

            def chain_pe_pipe(key, w=NBS):
                s2sb = chst[key]
                ncw = w // P
                colsp = ps.tile([P, NBS], F32, tag="pb", name=f"colsp{key}")
                for c in range(ncw):
                    nc.tensor.matmul(colsp[:, c:c + 1],
                                     s2sb[0:1, c * P:(c + 1) * P],
                                     cbf[0:1, 0:1], start=True, stop=True)
                nrm = smalls.tile([P, ncw], F32, tag=f"nrm{key}")
                nc.scalar.activation(nrm[:], colsp[:, 0:ncw], AF.Sqrt,
                                     bias=eps24_c[:, :])
                rsc = smalls.tile([P, ncw], F32, tag=f"rsc{key}")
                nc.vector.reciprocal(rsc[:], nrm[:])
                rsT = ps.tile([P, NBS], F32, tag="pb", name=f"rsT{key}")
                for c in range(ncw):
                    nc.tensor.matmul(rsT[0:1, c * P:(c + 1) * P],
                                     rsc[:, c:c + 1], cident[:],
                                     start=True, stop=True)
                rrow = rows.tile([1, NBS], BF16, tag="rrow",
                                 name=f"rrow{key}")
                nc.scalar.activation(rrow[:, 0:w], rsT[0:1, 0:w], AF.Copy)
                rb = ps.tile([P, NBS], F32, tag="pb", name=f"rsb{key}")
                # single outer-product mm: rb[i, j] = ones[i] * rrow[j]
                nc.tensor.matmul(rb[:, 0:w], bones[0:1, :], rrow[0:1, 0:w],
                                 start=True, stop=True)
                chst[key] = rb  # DVE reads the broadcast rs from PSUM

            def chain_vec(key, nb, c0, c1, scol):
                """g = relu(h)*rs (+BN sums) fused; g2 (+BN sumsq)."""
                rb = chst[key]
                w = c1 - c0
                for pt in range(PT):
                    gsl = g_sb[:, pt, nb * NBS + c0:nb * NBS + c1]
                    # relu commutes with the positive rs scale:
                    # g = max(h, 0) * rs, with the BN sum via accum.
                    # (scalar_tensor_tensor is DVE-only; Pool rejects it.)
                    nc.vector.scalar_tensor_tensor(
                        gsl, h_sb[(pt, nb)][:, c0:c1], 0.0, rb[:, 0:w],
                        op0=ALU.max, op1=ALU.mult,
                        accum_out=sums_g[:, pt, scol:scol + 1])
                    # g2 output is a throwaway (only accum_out matters)
                    g2t = work.tile([P, NBS], BF16, tag="wk",
                                    name=f"g2t{key}_{pt}")
                    if pt % 2 == 1:
                        nc.scalar.activation(
                            g2t[:, 0:w], gsl, AF.Square,
                            accum_out=sums_g2[:, pt, scol:scol + 1])
                    else:
                        nc.vector.scalar_tensor_tensor(
                            g2t[:, 0:w], gsl, 1.0, gsl, op0=ALU.mult,
                            op1=ALU.mult,
                            accum_out=sums_g2[:, pt, scol:scol + 1])

            # wave A: blocks 0..2 gather; chains emitted into wave B's stream
            gather_wave([0, 1, 2])

            # wave B: block 3, with wave-A chain PE ops interleaved so the
            # PE never stalls long on the vector chain
            hnpB = {oh: ps.tile([P, NBS], F32, tag="pb", name=f"hnpB{oh}")
                    for oh in range(OHALF)}
            nslB = slice(3 * NBS, 4 * NBS)
            for oh in range(OHALF):
                osl = slice(oh * P, (oh + 1) * P)
                for jc in range(JC):
                    nc.tensor.matmul(hnpB[oh][:], zh[:, jc, osl],
                                     at_sl(jc, nslB),
                                     start=(jc == 0), stop=(jc == JC - 1))
                    if oh == 0:
                        if jc == 4:
                            chain_pe_s2(0, 0, 0, NBS)
                        elif jc == 8:
                            chain_pe_s2(1, 1, 0, NBS)
                        elif jc == 10:
                            chain_pe_pipe(0)
                            chain_vec(0, 0, 0, NBS, 0)
                        elif jc == 13:
                            chain_pe_s2(2, 2, 0, NBS)
                        elif jc == 15:
                            chain_pe_pipe(1)
                            chain_vec(1, 1, 0, NBS, 1)
                    else:
                        if jc == 5:
                            chain_pe_pipe(2)
                            chain_vec(2, 2, 0, NBS, 2)
                # copies + squares for block 3's halves
                pt = 2 + oh
                ht = htiles.tile([P, NBS], BF16, tag="h", name=f"htn{oh}_3")
                h_sb[(pt, 3)] = ht
                nc.scalar.activation(ht[:], hnpB[oh][:], AF.Identity,
                                     bias=bias_c[:, pt:pt + 1])
                sqt = sqp.tile([P, NBS], BF16, tag="sq", name=f"sq3_{pt}")
                sq[(3, pt)] = sqt
                nc.vector.tensor_mul(sqt[:], ht[:], ht[:])
                if pt == 3:
                    nc.vector.tensor_add(sq[(3, 2)][:], sq[(3, 2)][:],
                                         sqt[:])

            # tail chain for block 3: two pipelined 256-col half-chains so
            # each serial stage is half-length
            H2 = NBS // 2
            chain_pe_s2("3a", 3, 0, H2)
            chain_pe_s2("3b", 3, H2, NBS)
            chain_pe_pipe("3a", H2)
            chain_vec("3a", 3, 0, H2, 3)
            chain_pe_pipe("3b", H2)
            chain_vec("3b", 3, H2, NBS, 4)

            # ---- phase 4: all-reduce BN stats ----
            stats_sb = smalls.tile([P, 2 * PT], F32)
            nc.vector.reduce_sum(stats_sb[:, 0:PT], sums_g[:],
                                 axis=mybir.AxisListType.X)
            nc.vector.reduce_sum(stats_sb[:, PT:2 * PT], sums_g2[:],
                                 axis=mybir.AxisListType.X)
            # AllGather + local tree-add: the gather is a single exchange
            # stage (vs the multi-stage mesh reduce) and the 3 adds are tiny
            stats_in = dram.tile([P, 2 * PT], F32)
            stats_ag = dram.tile([CORES * P, 2 * PT], F32)
            nc.sync.dma_start(stats_in[:], stats_sb[:])
            nc.gpsimd.collective_compute(
                "AllGather", ALU.bypass,
                replica_groups=[list(range(CORES))],
                ins=[stats_in.opt()],
                outs=[stats_ag.opt()],
            )
            allst = smalls.tile([P, CORES, 2 * PT], F32)
            nc.sync.dma_start(
                allst[:], stats_ag[:].rearrange("(c p) w -> p c w", p=P))
            r4 = smalls.tile([P, 4, 2 * PT], F32)
            nc.vector.tensor_tensor(r4[:], allst[:, 0:4, :],
                                    allst[:, 4:8, :], op=ALU.add)
            r2 = smalls.tile([P, 2, 2 * PT], F32)
            nc.vector.tensor_tensor(r2[:], r4[:, 0:2, :], r4[:, 2:4, :],
                                    op=ALU.add)
            allstats = smalls.tile([P, 2 * PT], F32)
            nc.vector.tensor_tensor(allstats[:], r2[:, 0, :], r2[:, 1, :],
                                    op=ALU.add)

            # mu/var -> per-channel scale/bias columns
            inv_n = 1.0 / (B * N)
            var_c = smalls.tile([P, PT], F32)
            s_c = smalls.tile([P, PT], F32)
            t_c = smalls.tile([P, PT], F32)
            mom_c = smalls.tile([P, 2 * PT], F32)
            nc.vector.tensor_scalar(mom_c[:], allstats[:], inv_n, None,
                                    op0=ALU.mult)
            mu_c = mom_c[:, 0:PT]
            ex2_c = mom_c[:, PT:2 * PT]
            nc.vector.tensor_mul(var_c[:], mu_c, mu_c)
            nc.vector.tensor_tensor(var_c[:], ex2_c, var_c[:],
                                    op=ALU.subtract)
            nc.scalar.activation(var_c[:], var_c[:], AF.Sqrt,
                                 bias=epsbn_c[:, :])
            nc.vector.reciprocal(var_c[:], var_c[:])
            nc.vector.tensor_mul(s_c[:], gam_c[:], var_c[:])
            nc.vector.tensor_mul(t_c[:], mu_c[:], s_c[:])
            nc.vector.tensor_tensor(t_c[:], bet_c[:], t_c[:], op=ALU.subtract)

            # ---- phase 5: y = g * s + t, store (wide tiles, 3 queues) ----
            N2 = N // 2
            dmaq = [nc.sync, nc.gpsimd, nc.scalar]
            k = 0
            for pt in range(PT):
                for half in range(2):
                    yt = yst.tile([P, N2], BF16, tag="y")
                    gsl = g_sb[:, pt, half * N2:(half + 1) * N2]
                    if k % 2 == 0:
                        nc.scalar.activation(
                            yt[:], gsl, AF.Identity, bias=t_c[:, pt:pt + 1],
                            scale=s_c[:, pt:pt + 1])
                    else:
                        nc.vector.tensor_scalar(
                            yt[:], gsl, s_c[:, pt:pt + 1], t_c[:, pt:pt + 1],
                            op0=ALU.mult, op1=ALU.add)
                    dmaq[k % 3].dma_start(
                        y_d[pt * P:(pt + 1) * P, half * N2:(half + 1) * N2],
                        yt[:])
                    k += 1

    nc.compile()
    return nc


def _prepare_inputs(x, idx_neib, Wx_w, Wx_b, Wn_w, Wn_b, gamma, beta):
    x = np.asarray(x, dtype=np.float32)
    idx = np.asarray(idx_neib, dtype=np.int64)
    Wx_w = np.asarray(Wx_w, dtype=np.float32)
    Wn_w = np.asarray(Wn_w, dtype=np.float32)
    Wx_b = np.asarray(Wx_b, dtype=np.float32)
    Wn_b = np.asarray(Wn_b, dtype=np.float32)
    gamma = np.asarray(gamma, dtype=np.float32)
    beta = np.asarray(beta, dtype=np.float32)

    # adjacency counts: AT[j, n] = #{k : idx[n, k] == j}
    at = np.zeros((N, N), dtype=np.float32)
    np.add.at(at, (idx.ravel(), np.repeat(np.arange(N), K)), 1.0)
    assert at.max() <= 16, "neighbor multiplicity too large for fp8 counts"
    at_q = at.astype(mybir.dt.np(A_DT))

    wx = np.ascontiguousarray(Wx_w.T).astype(ml_dtypes.bfloat16)
    wn = (np.ascontiguousarray(Wn_w.T) / np.float32(K)).astype(
        ml_dtypes.bfloat16)
    wpk = np.ascontiguousarray(np.stack([wx, wn], axis=1))

    cpk = np.zeros((P, CPK_W), dtype=np.float32)
    cpk[:, CPK_BIAS + 0] = Wx_b[0:P]
    cpk[:, CPK_BIAS + 1] = Wx_b[P:O]
    cpk[:, CPK_BIAS + 2] = Wn_b[0:P]
    cpk[:, CPK_BIAS + 3] = Wn_b[P:O]
    cpk[:, CPK_GAM:CPK_GAM + PT] = gamma.reshape(PT, P).T
    cpk[:, CPK_BET:CPK_BET + PT] = beta.reshape(PT, P).T
    cpk[:, CPK_EPS24] = 1e-24
    cpk[:, CPK_EPSBN] = 1e-5
    cpk[:, CPK_ONES:CPK_ONES + P] = 1.0
    cpk[:, CPK_ID:CPK_ID + P] = np.eye(P, dtype=np.float32)

    shared = dict(AT=at_q, wpk=wpk, cpk=cpk)
    in_maps = []
    for b in range(B):
        m = dict(shared)
        m["xt"] = np.ascontiguousarray(x[b].T).astype(ml_dtypes.bfloat16)
        in_maps.append(m)
    return in_maps


def kernel(x, idx_neib, Wx_w, Wx_b, Wn_w, Wn_b, gamma, beta, _trace=False,
           _trace_cores=None):
    if "nc" not in _cache:
        _cache["nc"] = build_program()
    nc = _cache["nc"]
    in_maps = _prepare_inputs(x, idx_neib, Wx_w, Wx_b, Wn_w, Wn_b, gamma, beta)
    res = run_bass_kernel_spmd(nc, in_maps, list(range(CORES)), trace=_trace,
                               trace_cores=_trace_cores)
    _cache["last_results"] = res
    y = np.stack([np.asarray(res.results[c]["y"]).astype(np.float32)
                  for c in range(CORES)])
    return np.ascontiguousarray(y.transpose(0, 2, 1))  # [B, N, CH]
